# revision 10
# baseline (speedup 1.0000x reference)
import numpy as np

# nn_Head: single-head causal attention.
# B=8, T=2048, E=1024, D=128. Data-parallel: one batch element per core.
# Per core: q/k/v projections (bf16 matmuls), causal softmax(q k^T / sqrt(D)) @ v.
#
# Layout trick: compute S^T = K @ q^T directly ([key, query]); then
# P^T = exp(S^T) is exactly the stationary (lhsT) operand needed by the
# P @ V matmul, so no PE transposes are needed. The softmax row-sum is
# obtained for free by appending a ones-column to V (N = D+1 = 129), and
# the max-subtraction is skipped (scores are O(1), exp is safe in fp32).
B, T, E, D = 8, 2048, 1024, 128
SCALE = 1.0 / np.sqrt(D)
NT = T // 128        # 16 query/key row tiles
NE = E // 128        # 8 contraction chunks
NC_CHUNK = T // 512  # 4 query chunks of 512


def _build():
    from concourse import bacc, bass, tile
    from concourse.bass import mybir

    f32 = mybir.dt.float32
    bf16 = mybir.dt.bfloat16
    nc = bacc.Bacc(None, target_bir_lowering=False)

    XT_d = nc.declare_dram_parameter("XT", [E, T], bf16, isOutput=False)
    Wq_d = nc.declare_dram_parameter("Wq", [E, D], bf16, isOutput=False)
    Wk_d = nc.declare_dram_parameter("Wk", [E, D], bf16, isOutput=False)
    Wv_d = nc.declare_dram_parameter("Wv", [E, D], bf16, isOutput=False)
    maskT_d = nc.declare_dram_parameter("maskT", [128, 128], f32, isOutput=False)
    out_d = nc.declare_dram_parameter("out", [T, D], bf16, isOutput=True)

    with tile.TileContext(nc) as tc:
        with (
            tc.tile_pool(name="persist", bufs=1) as pp,
            tc.tile_pool(name="pt", bufs=8) as ptp,
            tc.tile_pool(name="ostage", bufs=3) as wp,
            tc.tile_pool(name="rstage", bufs=3) as rp,
            tc.tile_pool(name="spsum", bufs=4, space=bass.MemorySpace.PSUM) as sp,
            tc.tile_pool(name="apsum", bufs=4, space=bass.MemorySpace.PSUM) as ap,
        ):
            XT = pp.tile([128, NE, T], bf16)      # X^T: [e, t]
            Wq = pp.tile([128, NE, D], bf16)
            Wk = pp.tile([128, NE, D], bf16)
            Wv = pp.tile([128, NE, D], bf16)
            qT = pp.tile([128, T], bf16)          # q^T [d, t]
            kT = pp.tile([128, T], bf16)          # k^T [d, t]
            v = pp.tile([128, NT, D + 1], bf16)   # v [t, d] row-tiled, col D = ones
            maskT = pp.tile([128, 128], f32)      # [k, q]: -1e30 where k > q

            nc.gpsimd.dma_start(maskT[:], maskT_d[:])
            for e in range(NE):
                nc.gpsimd.dma_start(XT[:, e, :], XT_d[e * 128:(e + 1) * 128, :])
                nc.gpsimd.dma_start(Wq[:, e, :], Wq_d[e * 128:(e + 1) * 128, :])
                nc.gpsimd.dma_start(Wk[:, e, :], Wk_d[e * 128:(e + 1) * 128, :])
                nc.gpsimd.dma_start(Wv[:, e, :], Wv_d[e * 128:(e + 1) * 128, :])

            nc.vector.memset(v[:, :, D:D + 1], 1.0)

            Exp = mybir.ActivationFunctionType.Exp
            Add = mybir.AluOpType.add

            def emit_S(qc, j):
                # S^T block [key tile j (128), query chunk qc (512)] -> P^T bf16
                live0 = max(0, j - 4 * qc) * 128
                q0 = qc * 512
                S = sp.tile([128, 512], f32, name="S")
                nc.tensor.matmul(
                    S[:, live0:512],
                    kT[:, j * 128:(j + 1) * 128],
                    qT[:, q0 + live0:q0 + 512],
                    start=True, stop=True)
                if j >= 4 * qc:  # diagonal block: causal mask
                    nc.vector.tensor_tensor(
                        S[:, live0:live0 + 128], S[:, live0:live0 + 128],
                        maskT[:], op=Add)
                Pt = ptp.tile([128, 512], bf16, name="Pt")
                nc.scalar.activation(
                    Pt[:, live0:512], S[:, live0:512], Exp, bias=0.0, scale=SCALE)
                return Pt

            # q/k projections: qT/kT [d, t] = W^T @ X^T, 512-wide t chunks
            for c in range(NC_CHUNK):
                for W, dst in ((Wq, qT), (Wk, kT)):
                    ps = sp.tile([128, 512], f32, name="S")
                    for e in range(NE):
                        nc.tensor.matmul(
                            ps[:], W[:, e, :], XT[:, e, c * 512:(c + 1) * 512],
                            start=(e == 0), stop=(e == NE - 1))
                    nc.vector.tensor_copy(dst[:, c * 512:(c + 1) * 512], ps[:])
                if c == 0:
                    # queries 0..511 attend only to keys 0..511: S^T for
                    # chunk 0 can run now, letting exp overlap the v phase.
                    pts0 = [emit_S(0, j) for j in range(4)]

            # v: [t, d] = X @ Wv, one 128-row tile at a time
            for t in range(NT):
                ps = ap.tile([128, 512], f32, name="acc")
                for e in range(NE):
                    nc.tensor.matmul(
                        ps[:, 0:D], XT[:, e, t * 128:(t + 1) * 128], Wv[:, e, :],
                        start=(e == 0), stop=(e == NE - 1))
                nc.vector.tensor_copy(v[:, t, 0:D], ps[:, 0:D])

            # attention: per query chunk, accumulate P @ [V | 1] over key tiles
            for qc in range(NC_CHUNK):
                nj = 4 * qc + 4
                accs = [ap.tile([128, 512], f32, name="acc") for i in range(4)]
                if qc == 0:
                    pts = pts0
                else:
                    pts = [None] * nj
                    pts[0] = emit_S(qc, 0)
                    pts[1] = emit_S(qc, 1)
                for j in range(nj):
                    if qc > 0 and j + 2 < nj:
                        pts[j + 2] = emit_S(qc, j + 2)
                    for tl in range(4):
                        tg = 4 * qc + tl
                        if tg < j:
                            continue
                        nc.tensor.matmul(
                            accs[tl][:, 0:D + 1],
                            pts[j][:, tl * 128:(tl + 1) * 128],
                            v[:, j, 0:D + 1],
                            start=(j == 0), stop=(j == tg))
                        if j == tg:
                            rcp = rp.tile([128, 1], f32)
                            nc.vector.reciprocal(rcp[:], accs[tl][:, D:D + 1])
                            o = wp.tile([128, D], bf16)
                            nc.vector.tensor_scalar_mul(
                                o[:], accs[tl][:, 0:D], rcp[:])
                            nc.gpsimd.dma_start(
                                out_d[tg * 128:(tg + 1) * 128, :], o[:])

    nc.compile()
    return nc


_NC = None
LAST_RESULT = None


def kernel(X, Wq, Wk, Wv):
    global _NC, LAST_RESULT
    import ml_dtypes
    from concourse.bass_utils import run_bass_kernel_spmd

    if _NC is None:
        _NC = _build()
    bf = ml_dtypes.bfloat16
    Xb = np.asarray(X, np.float32).astype(bf)            # [B, T, E]
    XTb = np.ascontiguousarray(Xb.transpose(0, 2, 1))    # [B, E, T]
    maskT = np.tril(np.full((128, 128), -1e30, np.float32), -1)
    base = {
        "Wq": np.ascontiguousarray(np.asarray(Wq, np.float32).astype(bf)),
        "Wk": np.ascontiguousarray(np.asarray(Wk, np.float32).astype(bf)),
        "Wv": np.ascontiguousarray(np.asarray(Wv, np.float32).astype(bf)),
        "maskT": maskT,
    }
    in_maps = [dict(base, XT=XTb[b]) for b in range(B)]
    res = run_bass_kernel_spmd(_NC, in_maps, core_ids=list(range(B)))
    LAST_RESULT = res
    outs = []
    for r in res.results:
        o = np.asarray(r["out"] if isinstance(r, dict) else r)
        outs.append(o.astype(np.float32))
    return np.stack(outs, 0).reshape(B, T, D)
